# revision 27
# baseline (speedup 1.0000x reference)
"""CompressAttn Trainium2 Bass kernel (v2).

Problem: compressed-block attention.
  B=2, N=4096, QH=32, KH=2, D=VD=128, KSZ=32, STRIDE=16, M=255 blocks.
  kc[b,m,h,:] = sum_i w_k[i] * (k[b,16m+i,h,:] + pe_k[i,:])   (same for v)
  out = softmax(q @ kc^T * D^-0.5, causal-banded mask) @ vc, zero for n < 31.

Sharding: 8 cores = (batch b in {0,1}) x (query-head quarter hq in {0..3}).
Each core handles 8 query heads that share a single KV head, so K/V
compression is done once per core.  No collectives; host gathers.

Per-core device pipeline (v2 — fat ops, mask on PE, host-side normalize):
  1. Compression via banded matmul (fp32r), producing kcT [d, m] (bf16) and
     vca0/vca1 [m-chunk, vc|1|0] (bf16, ones col 128 gives softmax denom).
  2. Per (head, 512-query block): sT psum [128, 1024] = chunk0|chunk1 scores;
     the causal staircase mask is ADDED on the tensor engine as an extra
     accumulating matmul: shifted-identity (stationary) x staircase constant
     of -1e30/0 (moving) lands exactly on the 64-row diagonal window.
  3. One ScalarE exp over the whole [128, 1024] psum span -> eT bf16 SBUF.
  4. PV per 128-query tile: psum[128,130x2] = eT_tile^T @ [vc|1|0], 1-2
     accumulating matmuls; embedded ones column yields the denominator.
  5. DVE evacuates numerator|denominator to bf16 SBUF; one DMA per head.
     Softmax division num/den happens on the host during the gather.
"""

import ml_dtypes
import numpy as np

import concourse.bacc as bacc
import concourse.mybir as mybir
import concourse.tile as tile
from concourse.bass_utils import run_bass_kernel_spmd

# Problem geometry (hardcoded per contest rules).
B, N, QH, KH, D, VD = 2, 4096, 32, 2, 128, 128
KSZ, STRIDE = 32, 16
M = (N - KSZ) // STRIDE + 1          # 255 compressed blocks
HPC = QH // 4                         # 8 query heads per core
NBLK = N // 512                       # 8 query blocks of 512
SM = float(D) ** -0.5
NEG = -1e30

F32 = mybir.dt.float32
F32R = mybir.dt.float32r
BF16 = mybir.dt.bfloat16

USE_PE_MASK = False  # True: staircase mask as PE accumulate-matmul; False: DVE mul


def build_program():
    nc = bacc.Bacc("TRN2", target_bir_lowering=False, debug=False)

    qT_d = nc.dram_tensor("qT", [HPC, D, N], BF16, kind="ExternalInput")
    k_d = nc.dram_tensor("kk", [128, 4096], BF16, kind="ExternalInput")
    v_d = nc.dram_tensor("vv", [128, 4096], BF16, kind="ExternalInput")
    w01k_d = nc.dram_tensor("w01k", [128, 16], BF16, kind="ExternalInput")
    w01v_d = nc.dram_tensor("w01v", [128, 16], BF16, kind="ExternalInput")
    bk_d = nc.dram_tensor("biask", [128, 1], F32, kind="ExternalInput")
    bv_d = nc.dram_tensor("biasv", [128, 1], F32, kind="ExternalInput")
    emat_d = nc.dram_tensor("emat", [64, 256], BF16, kind="ExternalInput")
    stA_d = nc.dram_tensor("stairA", [64, 512], BF16, kind="ExternalInput")
    stB_d = nc.dram_tensor("stairB", [32, 512], BF16, kind="ExternalInput")
    m01_d = nc.dram_tensor("m01", [8, 128, 512], BF16, kind="ExternalInput")
    id_d = nc.dram_tensor("ident", [128, 128], F32, kind="ExternalInput")
    ones_d = nc.dram_tensor("ones1", [128, 2], BF16, kind="ExternalInput")
    # out: numerator cols 0:128 and denominator col 128 per 130-col group
    o_d = nc.dram_tensor("o", [HPC, NBLK, 128, 520], BF16, kind="ExternalOutput")

    with tile.TileContext(nc) as tc:
        with tc.tile_pool(name="consts", bufs=1) as cp:
            w01k = cp.tile([128, 16], BF16)
            w01v = cp.tile([128, 16], BF16)
            biask = cp.tile([128, 1], F32)
            biasv = cp.tile([128, 1], F32)
            emat = cp.tile([64, 256], BF16)
            stairA = cp.tile([64, 512], BF16)
            stairB = cp.tile([32, 512], BF16)
            m01 = cp.tile([128, 8 * 512], BF16)
            ident = cp.tile([128, 128], F32)
            ktile = cp.tile([128, 32 * 128], BF16)
            vtile = cp.tile([128, 32 * 128], BF16)
            kcT = cp.tile([128, 256], BF16)       # [d, m] (col 255 zero-pad)
            vcT = cp.tile([128, 256], F32)        # [d, t] staging
            vca0 = cp.tile([128, 130], BF16)      # [m 0:128,   vc|1|0]
            vca1 = cp.tile([128, 130], BF16)      # [m 128:255, vc|1|0]

            # DMA ring plan: sync = smalls + k + qT(head0); gpsimd = v;
            # scalar = m01 then per-head qT prefetch.  Rings run in parallel
            # at ~150 GB/s each and process their queue in order.
            nc.sync.dma_start(w01k[:, :], w01k_d.ap())
            nc.sync.dma_start(w01v[:, :], w01v_d.ap())
            nc.sync.dma_start(biask[:, :], bk_d.ap())
            nc.sync.dma_start(biasv[:, :], bv_d.ap())
            nc.sync.dma_start(ident[:, :], id_d.ap())
            nc.sync.dma_start(vca0[:, 128:130], ones_d.ap())
            nc.sync.dma_start(vca1[:, 128:130], ones_d.ap())
            for cc in range(4):
                nc.sync.dma_start(
                    ktile[:, 1024 * cc : 1024 * (cc + 1)],
                    k_d.ap()[:, 1024 * cc : 1024 * (cc + 1)],
                )
                nc.gpsimd.dma_start(
                    vtile[:, 1024 * cc : 1024 * (cc + 1)],
                    v_d.ap()[:, 1024 * cc : 1024 * (cc + 1)],
                )
            nc.scalar.dma_start(
                m01[:, :].rearrange("p (j n) -> p j n", j=8),
                m01_d.ap().rearrange("j p n -> p j n"),
            )

            # ---- compression ----
            with tc.tile_pool(name="ppsum", bufs=1, space="PSUM") as pp:
                # free layout (t, a): pkT[d, 2t+a] = P_a[t]
                pkT = pp.tile([128, 512], F32)
                pvT = pp.tile([128, 512], F32)
                tpA = pp.tile([128, 128], F32)
                tpB = pp.tile([128, 128], F32)
                for c in range(32):
                    nc.tensor.matmul(
                        pkT[:, 16 * c : 16 * c + 16],
                        ktile[:, 128 * c : 128 * (c + 1)],
                        w01k[:, :],
                        start=True, stop=True,
                    )
                for c in range(32):
                    nc.tensor.matmul(
                        pvT[:, 16 * c : 16 * c + 16],
                        vtile[:, 128 * c : 128 * (c + 1)],
                        w01v[:, :],
                        start=True, stop=True,
                    )
                # kcT[d,m] = P0[m] + P1[m+1] + bias_k[d]
                nc.vector.memset(kcT[:, M:256], 0.0)
                pk3 = pkT[:, :].rearrange("p (t a) -> p t a", a=2)
                pv3 = pvT[:, :].rearrange("p (t a) -> p t a", a=2)
                # (walrus: only one PSUM input per DVE op -> two steps)
                nc.vector.tensor_scalar_add(kcT[:, 0:M], pk3[:, 0:M, 0], biask[:, 0:1])
                nc.vector.tensor_add(kcT[:, 0:M], kcT[:, 0:M], pk3[:, 1 : M + 1, 1])
                nc.vector.tensor_scalar_add(vcT[:, 0:M], pv3[:, 0:M, 0], biasv[:, 0:1])
                nc.vector.tensor_add(vcT[:, 0:M], vcT[:, 0:M], pv3[:, 1 : M + 1, 1])
                nc.vector.memset(vcT[:, M : M + 1], 0.0)
                # transpose vcT -> natural vc, build [vc|1|0]
                nc.tensor.transpose(tpA[:, :], vcT[:, 0:128], ident[:, :])
                nc.tensor.transpose(tpB[:, :], vcT[:, 128:256], ident[:, :])
                nc.vector.tensor_copy(vca0[:, 0:128], tpA[:, :])
                nc.vector.tensor_copy(vca1[:, 0:128], tpB[:, :])

            # ---- attention ----
            with (
                tc.tile_pool(name="qp", bufs=2) as qp,
                tc.tile_pool(name="ep", bufs=4) as ep,
                tc.tile_pool(name="op", bufs=2) as op,
                tc.tile_pool(name="sps", bufs=2, space="PSUM") as sps,
                tc.tile_pool(name="pvs", bufs=4, space="PSUM") as pvs,
            ):
                qths = {}

                def load_q(hh):
                    if hh < HPC and hh not in qths:
                        t = qp.tile([128, N], BF16, tag="qTh", name=f"qTh{hh}")
                        if hh == 0:
                            for cc in range(4):
                                nc.sync.dma_start(
                                    t[:, 1024 * cc : 1024 * (cc + 1)],
                                    qT_d.ap()[0, :, 1024 * cc : 1024 * (cc + 1)],
                                )
                        else:
                            nc.scalar.dma_start(t[:, :], qT_d.ap()[hh])
                        qths[hh] = t

                load_q(0)
                for h in range(HPC):
                    load_q(h + 1)
                    qTh = qths.pop(h)
                    oH = op.tile([128, NBLK * 520], BF16, tag="oH")
                    for b in range(NBLK):
                        mr = min(32 * b + 31, M)      # visible m count
                        c0r = min(mr, 128)
                        c1r = mr - 128
                        w0 = 32 * b - 32              # mask window start (m)
                        qs = qTh[:, 512 * b : 512 * (b + 1)]

                        sT = sps.tile([128, 1024], F32, tag="sT")
                        # scores + staircase mask, one accumulation group per
                        # m-chunk.  The mask matmul adds stair[r, :] to score
                        # row (w0 + r) via a shifted identity stationary:
                        # emat[p, c] = (c == p + 96).
                        for ch in range(2):
                            cr = c0r if ch == 0 else c1r
                            if cr <= 0:
                                continue
                            mbase = w0 - 128 * ch
                            lo = max(mbase, 0)
                            hi = min(mbase + 64, 128)
                            have_mask = USE_PE_MASK and lo < hi
                            # full 128-col stationary (zero-padded kcT) so
                            # FWL engages; surplus rows are never read
                            nc.tensor.matmul(
                                sT[:, 512 * ch : 512 * (ch + 1)],
                                kcT[:, 128 * ch : 128 * (ch + 1)],
                                qs,
                                start=True, stop=not have_mask,
                            )
                            if have_mask:
                                rs = lo - mbase       # 0 or 32
                                nrow = hi - lo
                                x0 = 96 - mbase - rs
                                st = (
                                    stairA[0:nrow, :]
                                    if rs == 0
                                    else stairB[0:nrow, :]
                                )
                                nc.tensor.matmul(
                                    sT[:, 512 * ch : 512 * (ch + 1)],
                                    emat[0:nrow, x0 : x0 + 128],
                                    st,
                                    start=False, stop=True,
                                    skip_group_check=True,
                                )
                        eT = ep.tile([128, 1024], BF16, tag="eT")
                        ecols = 1024 if c1r > 0 else 512
                        nc.scalar.activation(
                            eT[:, 0:ecols], sT[:, 0:ecols],
                            mybir.ActivationFunctionType.Exp, scale=SM,
                        )
                        if not USE_PE_MASK:
                            # multiplicative staircase mask on the 64-row
                            # diagonal window (32-row pieces)
                            for ww in (w0, w0 + 32):
                                s0, e0 = max(ww, 0), min(ww + 32, c0r)
                                if s0 < e0:
                                    mj = m01[:, 512 * b : 512 * (b + 1)]
                                    nc.vector.tensor_mul(
                                        eT[s0:e0, 0:512], eT[s0:e0, 0:512],
                                        mj[s0:e0, :],
                                    )
                                if c1r > 0:
                                    s1 = max(ww, 128) - 128
                                    e1 = min(ww + 32, 128 + c1r) - 128
                                    if s1 < e1:
                                        mj = m01[:, 512 * (b - 4) : 512 * (b - 3)]
                                        nc.vector.tensor_mul(
                                            eT[s1:e1, 512:1024],
                                            eT[s1:e1, 512:1024],
                                            mj[s1:e1, :],
                                        )
                        # PV: two psum tiles (banks) of 2 q-tiles each.
                        # Bank-alternating emission so no two consecutive
                        # matmuls touch the same psum region (the RMW drain
                        # of an accumulate pair would serialize them).
                        pvtA = pvs.tile([128, 260], F32, tag="pv")
                        pvtB = pvs.tile([128, 260], F32, tag="pv")
                        for j in range(2):
                            for pvt, tt in ((pvtA, j), (pvtB, 2 + j)):
                                K = 8 * (4 * b + tt) + 7
                                c0k = min(K, 128)
                                c1k = K - 128
                                nc.tensor.matmul(
                                    pvt[:, 130 * (tt % 2) : 130 * (tt % 2) + 130],
                                    eT[0:c0k, 128 * tt : 128 * (tt + 1)],
                                    vca0[0:c0k, :],
                                    start=True, stop=(c1k <= 0),
                                    skip_group_check=True,
                                )
                            for pvt, tt in ((pvtA, j), (pvtB, 2 + j)):
                                K = 8 * (4 * b + tt) + 7
                                c1k = K - 128
                                if c1k > 0:
                                    nc.tensor.matmul(
                                        pvt[:, 130 * (tt % 2) : 130 * (tt % 2) + 130],
                                        eT[0:c1k, 512 + 128 * tt : 512 + 128 * (tt + 1)],
                                        vca1[0:c1k, :],
                                        start=False, stop=True,
                                        skip_group_check=True,
                                    )
                        nc.vector.tensor_copy(
                            oH[:, 520 * b : 520 * b + 260], pvtA[:, :]
                        )
                        nc.scalar.copy(
                            oH[:, 520 * b + 260 : 520 * b + 520], pvtB[:, :]
                        )
                    # output DMAs; finer chunks on the last head so the
                    # tail drains with less serial transfer left
                    nq = 4 if h == HPC - 1 else 2
                    bw = NBLK // nq
                    for qq in range(nq):
                        nc.sync.dma_start(
                            o_d.ap()[h, bw * qq : bw * (qq + 1)].rearrange(
                                "b p g -> p b g"
                            ),
                            oH[:, 520 * bw * qq : 520 * bw * (qq + 1)].rearrange(
                                "p (b g) -> p b g", b=bw
                            ),
                        )
    nc.compile()
    return nc


def make_consts(w_k, pe_k, w_v, pe_v):
    """Host-side constant tensors fed to every core."""
    f = np.float32
    w01k = np.zeros((128, 16), f)
    w01v = np.zeros((128, 16), f)
    for r in range(128):
        j = r // 16
        s = r % 16
        for a in range(2):
            # column layout (j, a): col = 2*j + a, matching psum (t, a)
            w01k[r, 2 * j + a] = w_k[16 * a + s]
            w01v[r, 2 * j + a] = w_v[16 * a + s]
    biask = (w_k[:, None] * pe_k).sum(0).astype(f)[:, None]  # [128,1]
    biasv = (w_v[:, None] * pe_v).sum(0).astype(f)[:, None]
    # shifted identity for the mask matmul: emat[p, c] = (c == p + 96)
    emat = np.zeros((64, 256), f)
    for p in range(64):
        emat[p, p + 96] = 1.0
    # staircase: stairA[r, c] = NEG where score row (w0+r) col c is masked,
    # i.e. c < 16 r - 481 (b-independent).  stairB = stairA[32:].
    stairA = np.zeros((64, 512), f)
    for r in range(64):
        cut = 16 * r - 481
        if cut > 0:
            stairA[r, : min(cut, 512)] = NEG
    stairB = stairA[32:64]
    # multiplicative variant: m01[v, p] = stair row (p - 32v + 32) as 0/1
    m01 = np.ones((8, 128, 512), f)
    for vv in range(8):
        for p in range(128):
            r = p - 32 * vv + 32
            if 0 <= r < 64:
                lo = 16 * r - 481
                if lo >= 512:
                    m01[vv, p, :] = 0.0
                else:
                    m01[vv, p, : max(lo, 0)] = 0.0
    ident = np.eye(128, dtype=f)
    bf = ml_dtypes.bfloat16
    return {
        "w01k": np.ascontiguousarray(w01k).astype(bf),
        "w01v": np.ascontiguousarray(w01v).astype(bf),
        "biask": np.ascontiguousarray(biask),
        "biasv": np.ascontiguousarray(biasv),
        "emat": emat.astype(bf),
        "stairA": np.ascontiguousarray(stairA).astype(bf),
        "stairB": np.ascontiguousarray(stairB).astype(bf),
        "m01": m01.astype(bf),
        "ident": ident,
        "ones1": np.hstack([np.ones((128, 1)), np.zeros((128, 1))]).astype(bf),
    }


def make_in_map(q, k, v, consts, core):
    b, hq = core // 4, core % 4
    g = hq // 2
    qT = np.ascontiguousarray(
        q[b, :, 8 * hq : 8 * (hq + 1), :].transpose(1, 2, 0)
    ).astype(ml_dtypes.bfloat16)  # [8, D, N]
    return {
        "qT": qT,
        "kk": np.ascontiguousarray(
            k[b, :, g, :].reshape(32, 128, 128).transpose(1, 0, 2).reshape(128, 4096)
        ).astype(ml_dtypes.bfloat16),
        "vv": np.ascontiguousarray(
            v[b, :, g, :].reshape(32, 128, 128).transpose(1, 0, 2).reshape(128, 4096)
        ).astype(ml_dtypes.bfloat16),
        **consts,
    }


_CACHE = {}


def _compiled():
    if "nc" not in _CACHE:
        _CACHE["nc"] = build_program()
    return _CACHE["nc"]


def kernel(q, k, v, w_k, pe_k, w_v, pe_v, _trace=False, _trace_kwargs=None):
    q = np.asarray(q, np.float32)
    k = np.asarray(k, np.float32)
    v = np.asarray(v, np.float32)
    consts = make_consts(
        np.asarray(w_k, np.float32), np.asarray(pe_k, np.float32),
        np.asarray(w_v, np.float32), np.asarray(pe_v, np.float32),
    )
    nc = _compiled()
    in_maps = [make_in_map(q, k, v, consts, c) for c in range(8)]
    kw = {}
    if _trace:
        kw = {"trace": True, **(_trace_kwargs or {})}
    res = run_bass_kernel_spmd(nc, in_maps, core_ids=list(range(8)), **kw)
    out = np.empty((B, N, QH, VD), np.float32)
    for c in range(8):
        b, hq = c // 4, c % 4
        arr = np.asarray(res.results[c]["o"], dtype=np.float32)
        # [HPC, NBLK, 128, 4, 130]: num cols 0:128, den col 128
        arr = arr.reshape(HPC, NBLK, 128, 4, 130)
        num = arr[..., 0:128]
        den = np.maximum(arr[..., 128:129], 1e-30)
        y = num / den                       # [h, blk, p, tt, vd]
        # n = 512*blk + 128*tt + p  ->  [blk, tt, p, h, vd]
        y = y.transpose(1, 3, 2, 0, 4).reshape(N, HPC, VD)
        out[b, :, 8 * hq : 8 * (hq + 1), :] = y
    _CACHE["last_result"] = res
    return out


# revision 28
# speedup vs baseline: 1.0208x; 1.0208x over previous
"""CompressAttn Trainium2 Bass kernel (v2).

Problem: compressed-block attention.
  B=2, N=4096, QH=32, KH=2, D=VD=128, KSZ=32, STRIDE=16, M=255 blocks.
  kc[b,m,h,:] = sum_i w_k[i] * (k[b,16m+i,h,:] + pe_k[i,:])   (same for v)
  out = softmax(q @ kc^T * D^-0.5, causal-banded mask) @ vc, zero for n < 31.

Sharding: 8 cores = (batch b in {0,1}) x (query-head quarter hq in {0..3}).
Each core handles 8 query heads that share a single KV head, so K/V
compression is done once per core.  No collectives; host gathers.

Per-core device pipeline (v2 — fat ops, mask on PE, host-side normalize):
  1. Compression via banded matmul (fp32r), producing kcT [d, m] (bf16) and
     vca0/vca1 [m-chunk, vc|1|0] (bf16, ones col 128 gives softmax denom).
  2. Per (head, 512-query block): sT psum [128, 1024] = chunk0|chunk1 scores;
     the causal staircase mask is ADDED on the tensor engine as an extra
     accumulating matmul: shifted-identity (stationary) x staircase constant
     of -1e30/0 (moving) lands exactly on the 64-row diagonal window.
  3. One ScalarE exp over the whole [128, 1024] psum span -> eT bf16 SBUF.
  4. PV per 128-query tile: psum[128,130x2] = eT_tile^T @ [vc|1|0], 1-2
     accumulating matmuls; embedded ones column yields the denominator.
  5. DVE evacuates numerator|denominator to bf16 SBUF; one DMA per head.
     Softmax division num/den happens on the host during the gather.
"""

import ml_dtypes
import numpy as np

import concourse.bacc as bacc
import concourse.mybir as mybir
import concourse.tile as tile
from concourse.bass_utils import run_bass_kernel_spmd

# Problem geometry (hardcoded per contest rules).
B, N, QH, KH, D, VD = 2, 4096, 32, 2, 128, 128
KSZ, STRIDE = 32, 16
M = (N - KSZ) // STRIDE + 1          # 255 compressed blocks
HPC = QH // 4                         # 8 query heads per core
NBLK = N // 512                       # 8 query blocks of 512
SM = float(D) ** -0.5
NEG = -1e30

F32 = mybir.dt.float32
F32R = mybir.dt.float32r
BF16 = mybir.dt.bfloat16

USE_PE_MASK = False  # True: staircase mask as PE accumulate-matmul; False: DVE mul


def build_program():
    nc = bacc.Bacc("TRN2", target_bir_lowering=False, debug=False)

    qT_d = nc.dram_tensor("qT", [HPC, D, N], BF16, kind="ExternalInput")
    k_d = nc.dram_tensor("kk", [128, 4096], BF16, kind="ExternalInput")
    v_d = nc.dram_tensor("vv", [128, 4096], BF16, kind="ExternalInput")
    w01k_d = nc.dram_tensor("w01k", [128, 16], BF16, kind="ExternalInput")
    w01v_d = nc.dram_tensor("w01v", [128, 16], BF16, kind="ExternalInput")
    bk_d = nc.dram_tensor("biask", [128, 1], F32, kind="ExternalInput")
    bv_d = nc.dram_tensor("biasv", [128, 1], F32, kind="ExternalInput")
    emat_d = nc.dram_tensor("emat", [64, 256], BF16, kind="ExternalInput")
    stA_d = nc.dram_tensor("stairA", [64, 512], BF16, kind="ExternalInput")
    stB_d = nc.dram_tensor("stairB", [32, 512], BF16, kind="ExternalInput")
    m01_d = nc.dram_tensor("m01", [8, 128, 512], BF16, kind="ExternalInput")
    id_d = nc.dram_tensor("ident", [128, 128], F32, kind="ExternalInput")
    ones_d = nc.dram_tensor("ones1", [128, 2], BF16, kind="ExternalInput")
    # out: numerator cols 0:128 and denominator col 128 per 130-col group
    o_d = nc.dram_tensor("o", [HPC, NBLK, 128, 520], BF16, kind="ExternalOutput")

    with tile.TileContext(nc) as tc:
        with tc.tile_pool(name="consts", bufs=1) as cp:
            w01k = cp.tile([128, 16], BF16)
            w01v = cp.tile([128, 16], BF16)
            biask = cp.tile([128, 1], F32)
            biasv = cp.tile([128, 1], F32)
            emat = cp.tile([64, 256], BF16)
            stairA = cp.tile([64, 512], BF16)
            stairB = cp.tile([32, 512], BF16)
            m01 = cp.tile([128, 8 * 512], BF16)
            ident = cp.tile([128, 128], F32)
            ktile = cp.tile([128, 32 * 128], BF16)
            vtile = cp.tile([128, 32 * 128], BF16)
            kcT = cp.tile([128, 256], BF16)       # [d, m] (col 255 zero-pad)
            vcT = cp.tile([128, 256], F32)        # [d, t] staging
            vca0 = cp.tile([128, 130], BF16)      # [m 0:128,   vc|1|0]
            vca1 = cp.tile([128, 130], BF16)      # [m 128:255, vc|1|0]

            # DMA ring plan: sync = w01 + k + qT(head0) + smalls;
            # gpsimd = v; scalar = m01/per-head qT, gated behind k's
            # arrival so the HBM arbiter (FCFS, ~250 GB/s aggregate)
            # serves the critical path first.
            nc.sync.dma_start(w01k[:, :], w01k_d.ap())
            nc.sync.dma_start(w01v[:, :], w01v_d.ap())
            for cc in range(4):
                nc.sync.dma_start(
                    ktile[:, 1024 * cc : 1024 * (cc + 1)],
                    k_d.ap()[:, 1024 * cc : 1024 * (cc + 1)],
                )
                nc.gpsimd.dma_start(
                    vtile[:, 1024 * cc : 1024 * (cc + 1)],
                    v_d.ap()[:, 1024 * cc : 1024 * (cc + 1)],
                )
            nc.sync.dma_start(biask[:, :], bk_d.ap())
            nc.sync.dma_start(biasv[:, :], bv_d.ap())
            nc.sync.dma_start(ident[:, :], id_d.ap())
            nc.sync.dma_start(vca0[:, 128:130], ones_d.ap())
            nc.sync.dma_start(vca1[:, 128:130], ones_d.ap())
            # gate: this scalar op waits for k's last chunk, holding the
            # scalar ring's m01/qT triggers back until k has landed
            kgate = cp.tile([1, 16], BF16)
            nc.scalar.copy(kgate[0:1, :], ktile[0:1, 4080:4096])
            nc.scalar.dma_start(
                m01[:, :].rearrange("p (j n) -> p j n", j=8),
                m01_d.ap().rearrange("j p n -> p j n"),
            )

            # ---- compression ----
            with tc.tile_pool(name="ppsum", bufs=1, space="PSUM") as pp:
                # free layout (t, a): pkT[d, 2t+a] = P_a[t]
                pkT = pp.tile([128, 512], F32)
                pvT = pp.tile([128, 512], F32)
                tpA = pp.tile([128, 128], F32)
                tpB = pp.tile([128, 128], F32)
                for c in range(32):
                    nc.tensor.matmul(
                        pkT[:, 16 * c : 16 * c + 16],
                        ktile[:, 128 * c : 128 * (c + 1)],
                        w01k[:, :],
                        start=True, stop=True,
                    )
                for c in range(32):
                    nc.tensor.matmul(
                        pvT[:, 16 * c : 16 * c + 16],
                        vtile[:, 128 * c : 128 * (c + 1)],
                        w01v[:, :],
                        start=True, stop=True,
                    )
                # kcT[d,m] = P0[m] + P1[m+1] + bias_k[d]
                nc.vector.memset(kcT[:, M:256], 0.0)
                pk3 = pkT[:, :].rearrange("p (t a) -> p t a", a=2)
                pv3 = pvT[:, :].rearrange("p (t a) -> p t a", a=2)
                # (walrus: only one PSUM input per DVE op -> two steps)
                nc.vector.tensor_scalar_add(kcT[:, 0:M], pk3[:, 0:M, 0], biask[:, 0:1])
                nc.vector.tensor_add(kcT[:, 0:M], kcT[:, 0:M], pk3[:, 1 : M + 1, 1])
                nc.vector.tensor_scalar_add(vcT[:, 0:M], pv3[:, 0:M, 0], biasv[:, 0:1])
                nc.vector.tensor_add(vcT[:, 0:M], vcT[:, 0:M], pv3[:, 1 : M + 1, 1])
                nc.vector.memset(vcT[:, M : M + 1], 0.0)
                # transpose vcT -> natural vc, build [vc|1|0]
                nc.tensor.transpose(tpA[:, :], vcT[:, 0:128], ident[:, :])
                nc.tensor.transpose(tpB[:, :], vcT[:, 128:256], ident[:, :])
                nc.vector.tensor_copy(vca0[:, 0:128], tpA[:, :])
                nc.vector.tensor_copy(vca1[:, 0:128], tpB[:, :])

            # ---- attention ----
            with (
                tc.tile_pool(name="qp", bufs=2) as qp,
                tc.tile_pool(name="ep", bufs=4) as ep,
                tc.tile_pool(name="op", bufs=2) as op,
                tc.tile_pool(name="sps", bufs=2, space="PSUM") as sps,
                tc.tile_pool(name="pvs", bufs=4, space="PSUM") as pvs,
            ):
                qths = {}

                def load_q(hh):
                    if hh < HPC and hh not in qths:
                        t = qp.tile([128, N], BF16, tag="qTh", name=f"qTh{hh}")
                        if hh == 0:
                            for cc in range(4):
                                nc.sync.dma_start(
                                    t[:, 1024 * cc : 1024 * (cc + 1)],
                                    qT_d.ap()[0, :, 1024 * cc : 1024 * (cc + 1)],
                                )
                        else:
                            nc.scalar.dma_start(t[:, :], qT_d.ap()[hh])
                        qths[hh] = t

                load_q(0)
                for h in range(HPC):
                    load_q(h + 1)
                    qTh = qths.pop(h)
                    oH = op.tile([128, NBLK * 520], BF16, tag="oH")
                    for b in range(NBLK):
                        mr = min(32 * b + 31, M)      # visible m count
                        c0r = min(mr, 128)
                        c1r = mr - 128
                        w0 = 32 * b - 32              # mask window start (m)
                        qs = qTh[:, 512 * b : 512 * (b + 1)]

                        sT = sps.tile([128, 1024], F32, tag="sT")
                        # scores + staircase mask, one accumulation group per
                        # m-chunk.  The mask matmul adds stair[r, :] to score
                        # row (w0 + r) via a shifted identity stationary:
                        # emat[p, c] = (c == p + 96).
                        for ch in range(2):
                            cr = c0r if ch == 0 else c1r
                            if cr <= 0:
                                continue
                            mbase = w0 - 128 * ch
                            lo = max(mbase, 0)
                            hi = min(mbase + 64, 128)
                            have_mask = USE_PE_MASK and lo < hi
                            # full 128-col stationary (zero-padded kcT) so
                            # FWL engages; surplus rows are never read
                            nc.tensor.matmul(
                                sT[:, 512 * ch : 512 * (ch + 1)],
                                kcT[:, 128 * ch : 128 * (ch + 1)],
                                qs,
                                start=True, stop=not have_mask,
                            )
                            if have_mask:
                                rs = lo - mbase       # 0 or 32
                                nrow = hi - lo
                                x0 = 96 - mbase - rs
                                st = (
                                    stairA[0:nrow, :]
                                    if rs == 0
                                    else stairB[0:nrow, :]
                                )
                                nc.tensor.matmul(
                                    sT[:, 512 * ch : 512 * (ch + 1)],
                                    emat[0:nrow, x0 : x0 + 128],
                                    st,
                                    start=False, stop=True,
                                    skip_group_check=True,
                                )
                        eT = ep.tile([128, 1024], BF16, tag="eT")
                        ecols = 1024 if c1r > 0 else 512
                        nc.scalar.activation(
                            eT[:, 0:ecols], sT[:, 0:ecols],
                            mybir.ActivationFunctionType.Exp, scale=SM,
                        )
                        if not USE_PE_MASK:
                            # multiplicative staircase mask on the 64-row
                            # diagonal window (32-row pieces)
                            for ww in (w0, w0 + 32):
                                s0, e0 = max(ww, 0), min(ww + 32, c0r)
                                if s0 < e0:
                                    mj = m01[:, 512 * b : 512 * (b + 1)]
                                    nc.vector.tensor_mul(
                                        eT[s0:e0, 0:512], eT[s0:e0, 0:512],
                                        mj[s0:e0, :],
                                    )
                                if c1r > 0:
                                    s1 = max(ww, 128) - 128
                                    e1 = min(ww + 32, 128 + c1r) - 128
                                    if s1 < e1:
                                        mj = m01[:, 512 * (b - 4) : 512 * (b - 3)]
                                        nc.vector.tensor_mul(
                                            eT[s1:e1, 512:1024],
                                            eT[s1:e1, 512:1024],
                                            mj[s1:e1, :],
                                        )
                        # PV: two psum tiles (banks) of 2 q-tiles each.
                        # Bank-alternating emission so no two consecutive
                        # matmuls touch the same psum region (the RMW drain
                        # of an accumulate pair would serialize them).
                        pvtA = pvs.tile([128, 260], F32, tag="pv")
                        pvtB = pvs.tile([128, 260], F32, tag="pv")
                        for j in range(2):
                            for pvt, tt in ((pvtA, j), (pvtB, 2 + j)):
                                K = 8 * (4 * b + tt) + 7
                                c0k = min(K, 128)
                                c1k = K - 128
                                nc.tensor.matmul(
                                    pvt[:, 130 * (tt % 2) : 130 * (tt % 2) + 130],
                                    eT[0:c0k, 128 * tt : 128 * (tt + 1)],
                                    vca0[0:c0k, :],
                                    start=True, stop=(c1k <= 0),
                                    skip_group_check=True,
                                )
                            for pvt, tt in ((pvtA, j), (pvtB, 2 + j)):
                                K = 8 * (4 * b + tt) + 7
                                c1k = K - 128
                                if c1k > 0:
                                    nc.tensor.matmul(
                                        pvt[:, 130 * (tt % 2) : 130 * (tt % 2) + 130],
                                        eT[0:c1k, 512 + 128 * tt : 512 + 128 * (tt + 1)],
                                        vca1[0:c1k, :],
                                        start=False, stop=True,
                                        skip_group_check=True,
                                    )
                        nc.vector.tensor_copy(
                            oH[:, 520 * b : 520 * b + 260], pvtA[:, :]
                        )
                        nc.scalar.copy(
                            oH[:, 520 * b + 260 : 520 * b + 520], pvtB[:, :]
                        )
                    # output DMAs; finer chunks on the last head so the
                    # tail drains with less serial transfer left
                    nq = 4 if h == HPC - 1 else 2
                    bw = NBLK // nq
                    for qq in range(nq):
                        nc.sync.dma_start(
                            o_d.ap()[h, bw * qq : bw * (qq + 1)].rearrange(
                                "b p g -> p b g"
                            ),
                            oH[:, 520 * bw * qq : 520 * bw * (qq + 1)].rearrange(
                                "p (b g) -> p b g", b=bw
                            ),
                        )
    nc.compile()
    return nc


def make_consts(w_k, pe_k, w_v, pe_v):
    """Host-side constant tensors fed to every core."""
    f = np.float32
    w01k = np.zeros((128, 16), f)
    w01v = np.zeros((128, 16), f)
    for r in range(128):
        j = r // 16
        s = r % 16
        for a in range(2):
            # column layout (j, a): col = 2*j + a, matching psum (t, a)
            w01k[r, 2 * j + a] = w_k[16 * a + s]
            w01v[r, 2 * j + a] = w_v[16 * a + s]
    biask = (w_k[:, None] * pe_k).sum(0).astype(f)[:, None]  # [128,1]
    biasv = (w_v[:, None] * pe_v).sum(0).astype(f)[:, None]
    # shifted identity for the mask matmul: emat[p, c] = (c == p + 96)
    emat = np.zeros((64, 256), f)
    for p in range(64):
        emat[p, p + 96] = 1.0
    # staircase: stairA[r, c] = NEG where score row (w0+r) col c is masked,
    # i.e. c < 16 r - 481 (b-independent).  stairB = stairA[32:].
    stairA = np.zeros((64, 512), f)
    for r in range(64):
        cut = 16 * r - 481
        if cut > 0:
            stairA[r, : min(cut, 512)] = NEG
    stairB = stairA[32:64]
    # multiplicative variant: m01[v, p] = stair row (p - 32v + 32) as 0/1
    m01 = np.ones((8, 128, 512), f)
    for vv in range(8):
        for p in range(128):
            r = p - 32 * vv + 32
            if 0 <= r < 64:
                lo = 16 * r - 481
                if lo >= 512:
                    m01[vv, p, :] = 0.0
                else:
                    m01[vv, p, : max(lo, 0)] = 0.0
    ident = np.eye(128, dtype=f)
    bf = ml_dtypes.bfloat16
    return {
        "w01k": np.ascontiguousarray(w01k).astype(bf),
        "w01v": np.ascontiguousarray(w01v).astype(bf),
        "biask": np.ascontiguousarray(biask),
        "biasv": np.ascontiguousarray(biasv),
        "emat": emat.astype(bf),
        "stairA": np.ascontiguousarray(stairA).astype(bf),
        "stairB": np.ascontiguousarray(stairB).astype(bf),
        "m01": m01.astype(bf),
        "ident": ident,
        "ones1": np.hstack([np.ones((128, 1)), np.zeros((128, 1))]).astype(bf),
    }


def make_in_map(q, k, v, consts, core):
    b, hq = core // 4, core % 4
    g = hq // 2
    qT = np.ascontiguousarray(
        q[b, :, 8 * hq : 8 * (hq + 1), :].transpose(1, 2, 0)
    ).astype(ml_dtypes.bfloat16)  # [8, D, N]
    return {
        "qT": qT,
        "kk": np.ascontiguousarray(
            k[b, :, g, :].reshape(32, 128, 128).transpose(1, 0, 2).reshape(128, 4096)
        ).astype(ml_dtypes.bfloat16),
        "vv": np.ascontiguousarray(
            v[b, :, g, :].reshape(32, 128, 128).transpose(1, 0, 2).reshape(128, 4096)
        ).astype(ml_dtypes.bfloat16),
        **consts,
    }


_CACHE = {}


def _compiled():
    if "nc" not in _CACHE:
        _CACHE["nc"] = build_program()
    return _CACHE["nc"]


def kernel(q, k, v, w_k, pe_k, w_v, pe_v, _trace=False, _trace_kwargs=None):
    q = np.asarray(q, np.float32)
    k = np.asarray(k, np.float32)
    v = np.asarray(v, np.float32)
    consts = make_consts(
        np.asarray(w_k, np.float32), np.asarray(pe_k, np.float32),
        np.asarray(w_v, np.float32), np.asarray(pe_v, np.float32),
    )
    nc = _compiled()
    in_maps = [make_in_map(q, k, v, consts, c) for c in range(8)]
    kw = {}
    if _trace:
        kw = {"trace": True, **(_trace_kwargs or {})}
    res = run_bass_kernel_spmd(nc, in_maps, core_ids=list(range(8)), **kw)
    out = np.empty((B, N, QH, VD), np.float32)
    for c in range(8):
        b, hq = c // 4, c % 4
        arr = np.asarray(res.results[c]["o"], dtype=np.float32)
        # [HPC, NBLK, 128, 4, 130]: num cols 0:128, den col 128
        arr = arr.reshape(HPC, NBLK, 128, 4, 130)
        num = arr[..., 0:128]
        den = np.maximum(arr[..., 128:129], 1e-30)
        y = num / den                       # [h, blk, p, tt, vd]
        # n = 512*blk + 128*tt + p  ->  [blk, tt, p, h, vd]
        y = y.transpose(1, 3, 2, 0, 4).reshape(N, HPC, VD)
        out[b, :, 8 * hq : 8 * (hq + 1), :] = y
    _CACHE["last_result"] = res
    return out
